# revision 1
# baseline (speedup 1.0000x reference)
"""Trainium2 Bass kernel for nn_AudioRNN (LSTM(13->32, T=25) + FC(32->4), B=65536).

Strategy (pure data parallel over batch, 8 cores x 8192 rows):

  * Host side: x [B,25,13] is cast to bf16 and pre-transposed into the exact
    SBUF layout the TensorEngine needs, with a constant `ones` row appended so
    the LSTM bias rides along in the input-projection matmul.
  * Device side per core: batch is processed as `n_sc` "superchunks" of
    4*ch_b rows, each split into 4 chunks of ch_b rows.  Chunk c lives on
    SBUF/PSUM partition quadrant c (32 partitions = the 32 hidden dims), so
    all per-step tensors (gates, c, h) are lane-aligned for VectorE/ScalarE.
  * Gate pre-activations for one (t, superchunk) live in one PSUM tile
    [128, 4*ch_b]: free-dim bank G holds gate G (order f, i, o, g) so one
    Sigmoid op covers f,i,o and one Tanh covers g.
  * Matmuls run in 32x32 PE tiling: input projection uses tile rows = t%4
    (where the pre-transposed x lives), recurrence uses diagonal tiles (c,c),
    4 concurrent column tiles per wave.
"""

import numpy as np
import ml_dtypes

I_DIM = 13
H_DIM = 32
C_DIM = 4
T_STEPS = 25
B_FULL = 65536

N_TG = (T_STEPS + 3) // 4    # 7 t-groups of up to 4 timesteps
KX = I_DIM + 1               # 14: 13 input dims + ones row for bias

# free-dim bank order of the gates: f, i, o, g  (sigmoid on banks 0..2, tanh on 3)
# -> PyTorch row-chunk order in W_ih/W_hh is i(0), f(1), g(2), o(3)
GATE_PERM = [1, 0, 3, 2]     # bank G -> pytorch gate chunk index

# production config
N_CORES = 8
CH_B = 512                   # batch rows per chunk (= one PSUM bank of fp32)
N_SC = 4                     # superchunks per core

_BF16 = ml_dtypes.bfloat16

_NC_CACHE = {}


def _build_bass(n_sc=N_SC, ch_b=CH_B, split_waits=True, absorb=False, do_fc=True, do_store=True, do_recur=True, do_elem=True, recur_const=False, recur_diag=True, recur_acc=True):
    import concourse.bass as bass
    import concourse.mybir as mybir
    from concourse.tile import TileContext, add_dep_helper

    dt = mybir.dt
    AF = mybir.ActivationFunctionType

    sc_b = 4 * ch_b
    b_core = n_sc * sc_b

    nc = bass.Bass("TRN2")

    xt_d = nc.dram_tensor("xt", [n_sc, N_TG, 4, KX, sc_b], dt.bfloat16,
                          kind="ExternalInput")
    wx_d = nc.dram_tensor("wx", [4, KX, 128], dt.bfloat16, kind="ExternalInput")
    wh_d = nc.dram_tensor("wh", [4, H_DIM, 128], dt.bfloat16, kind="ExternalInput")
    wfc_d = nc.dram_tensor("wfc", [4, H_DIM, H_DIM], dt.bfloat16,
                           kind="ExternalInput")
    bfc_d = nc.dram_tensor("bfc", [128, 1], dt.float32, kind="ExternalInput")
    out_d = nc.dram_tensor("out", [b_core, C_DIM], dt.float32,
                           kind="ExternalOutput")

    c_dt = dt.bfloat16  # dtype of the cell state c

    with TileContext(nc) as tc:
        with (
            tc.tile_pool(name="singles", bufs=1) as singles,
            tc.tile_pool(name="xt", bufs=n_sc * N_TG) as xt_pool,
            tc.tile_pool(name="sig", bufs=6) as sig_pool,
            tc.tile_pool(name="cell", bufs=8) as cell_pool,
            tc.tile_pool(name="hid", bufs=8) as hid_pool,
            tc.tile_pool(name="tmp", bufs=8) as tmp_pool,
            tc.tile_pool(name="outp", bufs=4) as out_pool,
            tc.tile_pool(name="psum", bufs=2, space="PSUM") as psum_pool,
        ):
            # ---- constants / weights (replicated on all 4 partition strips)
            wx = singles.tile([128, 128], dt.bfloat16)
            wh = singles.tile([128, 128], dt.bfloat16)
            wfc = singles.tile([128, H_DIM], dt.bfloat16)
            bfc = singles.tile([128, 1], dt.float32)
            for u in range(4):
                nc.sync.dma_start(out=wx[32 * u:32 * u + KX, :], in_=wx_d[u])
                nc.sync.dma_start(out=wh[32 * u:32 * u + H_DIM, :], in_=wh_d[u])
                nc.sync.dma_start(out=wfc[32 * u:32 * u + H_DIM, :], in_=wfc_d[u])
            nc.sync.dma_start(out=bfc, in_=bfc_d[:, :])

            h_prev = [None] * n_sc
            c_prev = [None] * n_sc
            xt_cur = [None] * n_sc

            # History of PSUM "gates"-pool allocations: [readers, writer].
            # The MM ISA supports only ONE sync-wait command, but the first
            # matmul of each psum group naturally needs waits on (a) the
            # reused slot's prior ACT readers (WAR), (b) its prior PE writer
            # (WAW completion), (c) fresh DMA/DVE-produced operands.  We
            # absorb (a) and (b) onto zero-cost dummy LDWEIGHTS instructions
            # via explicit deps (pool bufs=2 -> reuse is 2 allocations ago),
            # and (c) onto another dummy LDW that references the operand.
            palloc_hist = []

            def absorb_slot_deps():
                if absorb and len(palloc_hist) >= 2:
                    old = palloc_hist[-2]
                    for r in old[0]:
                        ld = nc.tensor.ldweights(
                            weights=wx[0:32, 0:32], tile_position=(0, 0))
                        add_dep_helper(ld.ins, r.ins,
                                       reason="absorb psum-slot WAR")
                    if old[1] is not None:
                        ld = nc.tensor.ldweights(
                            weights=wx[0:32, 0:32], tile_position=(0, 0))
                        add_dep_helper(ld.ins, old[1].ins,
                                       reason="absorb psum-slot WAW")
                entry = [[], None]
                palloc_hist.append(entry)
                return entry

            rounds = [list(range(r, min(r + 2, n_sc)))
                      for r in range(0, n_sc, 2)]
            for rnd in rounds:
              for t in range(T_STEPS):
                tg, u = divmod(t, 4)
                for s in rnd:
                    # -- stage the pre-transposed x for this t-group
                    if u == 0:
                        xt = xt_pool.tile([128, sc_b], dt.bfloat16, tag="xt")
                        for uu in range(4):
                            if 4 * tg + uu < T_STEPS:
                                nc.sync.dma_start(
                                    out=xt[32 * uu:32 * uu + KX, :],
                                    in_=xt_d[s, tg, uu],
                                )
                        xt_cur[s] = xt
                    xr = xt_cur[s]

                    # -- gate pre-activations: one 4-bank PSUM tile
                    entry = absorb_slot_deps()
                    P = psum_pool.tile([128, 4 * ch_b], dt.float32, tag="gates")
                    if absorb:
                        # Absorb the xt DMA wait on a dummy LDW.
                        nc.tensor.ldweights(
                            weights=xr[32 * u:32 * u + 32, 0:32],
                            tile_position=(32 * u, 0),
                        )
                        if t > 0:
                            # Absorb the DVE wait for h on a dummy LDW too.
                            nc.tensor.ldweights(
                                weights=h_prev[s][0:32, 0:32],
                                tile_position=(0, 0),
                            )
                    last_mm = None
                    for g in range(4):
                        lx = wx[32 * u:32 * u + KX, 32 * g:32 * g + 32]
                        for c in range(4):
                            last_mm = nc.tensor.matmul(
                                out=P[32 * c:32 * c + 32,
                                      ch_b * g:ch_b * (g + 1)],
                                lhsT=lx,
                                rhs=xr[32 * u:32 * u + KX,
                                       ch_b * c:ch_b * (c + 1)],
                                start=True,
                                stop=(t == 0 or not do_recur),
                                skip_group_check=True,
                                tile_position=(32 * u, 32 * c),
                            )
                    if t > 0 and do_recur:
                        hp = xr if recur_const else h_prev[s]
                        for g in range(4):
                            for c in range(4):
                                rr = ((c + 1) % 4) if recur_diag == 2 else (c if recur_diag else u)
                                last_mm = nc.tensor.matmul(
                                    out=P[32 * c:32 * c + 32,
                                          ch_b * g:ch_b * (g + 1)],
                                    lhsT=wh[32 * rr:32 * rr + 32,
                                            32 * g:32 * g + 32],
                                    rhs=(hp[32 * rr:32 * rr + 32, 0:ch_b]
                                         if recur_const else
                                         hp[32 * rr:32 * rr + 32, :]),
                                    start=(not recur_acc),
                                    stop=True,
                                    skip_group_check=True,
                                    tile_position=(32 * rr, 32 * c),
                                )
                    entry[1] = last_mm

                    if not do_elem:
                        S0 = sig_pool.tile([128, 3 * ch_b], dt.bfloat16, tag="S")
                        rd = nc.scalar.activation(out=S0, in_=P[:, 0:3 * ch_b],
                                                  func=AF.Sigmoid)
                        entry[0] = [rd]
                        Hn0 = hid_pool.tile([128, ch_b], dt.bfloat16, tag="H")
                        nc.vector.tensor_copy(Hn0, S0[:, 0:ch_b])
                        c_prev[s] = None
                        h_prev[s] = Hn0
                        continue

                    # -- activations: sigmoid(f,i,o) in one op, tanh(g)
                    S = sig_pool.tile([128, 3 * ch_b], dt.bfloat16, tag="S")
                    sig_i = nc.scalar.activation(out=S, in_=P[:, 0:3 * ch_b],
                                                 func=AF.Sigmoid)
                    Gt = tmp_pool.tile([128, ch_b], dt.bfloat16, tag="Gt")
                    tg_i = nc.scalar.activation(out=Gt,
                                                in_=P[:, 3 * ch_b:4 * ch_b],
                                                func=AF.Tanh)
                    entry[0] = [sig_i, tg_i]

                    # -- cell update (all lane-aligned, 128 partitions busy)
                    Cn = cell_pool.tile([128, ch_b], c_dt, tag="C")
                    if t == 0:
                        nc.vector.tensor_mul(Cn, S[:, ch_b:2 * ch_b], Gt)
                    else:
                        FCt = tmp_pool.tile([128, ch_b], c_dt, tag="FCt")
                        IGt = tmp_pool.tile([128, ch_b], c_dt, tag="IGt")
                        nc.vector.tensor_mul(FCt, S[:, 0:ch_b], c_prev[s])
                        nc.vector.tensor_mul(IGt, S[:, ch_b:2 * ch_b], Gt)
                        nc.vector.tensor_add(Cn, FCt, IGt)
                    Tc = tmp_pool.tile([128, ch_b], dt.bfloat16, tag="Tc")
                    nc.scalar.activation(out=Tc, in_=Cn, func=AF.Tanh)
                    Hn = hid_pool.tile([128, ch_b], dt.bfloat16, tag="H")
                    nc.vector.tensor_mul(Hn, S[:, 2 * ch_b:3 * ch_b], Tc)
                    c_prev[s] = Cn
                    h_prev[s] = Hn

                    # -- final FC + bias + store, once per superchunk
                    if t == T_STEPS - 1 and do_fc:
                        fentry = absorb_slot_deps()
                        PF = psum_pool.tile([128, ch_b], dt.float32, tag="gates")
                        nc.tensor.ldweights(
                            weights=Hn[0:32, 0:32], tile_position=(0, 0),
                        )
                        for c in range(4):
                            # M=32 with zero-padded W_fc so the whole PSUM
                            # quadrant is written (no garbage reads below).
                            fentry[1] = nc.tensor.matmul(
                                out=PF[32 * c:32 * c + H_DIM, :],
                                lhsT=wfc[32 * c:32 * c + H_DIM, :],
                                rhs=Hn[32 * c:32 * c + H_DIM, :],
                                start=True,
                                stop=True,
                                skip_group_check=True,
                                tile_position=(32 * c, 32 * c),
                            )
                        Ot = out_pool.tile([128, ch_b], dt.float32, tag="O")
                        fentry[0] = [nc.scalar.add(Ot, PF, bfc)]
                        if not do_store:
                            continue
                        for c in range(4):
                            r0 = s * sc_b + c * ch_b
                            dst = out_d[r0:r0 + ch_b, :].rearrange("b m -> m b")
                            nc.gpsimd.dma_start(
                                out=dst, in_=Ot[32 * c:32 * c + C_DIM, :])

    if split_waits:
        _split_multi_waits(nc, mybir)
    return nc


def _split_multi_waits(nc, mybir):
    """This walrus build allows only ONE sync-wait command per ISA
    instruction.  Tile sometimes emits 2+ (its wait minimization is not
    transitive across processors).  Hoist all-but-one wait onto standalone
    EventSemaphore instructions injected just before, on the same engine —
    semantically identical (the engine stream blocks at the wait either way).
    """
    n_split = 0
    for fn in nc.m.functions:
        for blk in fn.blocks:
            out = []
            for inst in blk.instructions:
                si = getattr(inst, "sync_info", None)
                ow = list(si.on_wait) if si is not None and si.on_wait else []
                if len(ow) > 1 and inst.opcode == "DMACopy" \
                        and str(inst.engine) in ("EngineType.SP",
                                                 "EngineType.Activation"):
                    raise RuntimeError(
                        f"HWDGE DMA {inst.name} has {len(ow)} waits; "
                        "descriptor waits cannot be split safely")
                if len(ow) > 1:
                    for w in ow[:-1]:
                        n_split += 1
                        ev = mybir.InstEventSemaphore(
                            name=f"splitw-{n_split}-{inst.name}",
                            engine=inst.engine,
                            ins=[],
                            outs=[],
                            sync_info=mybir.SyncInfo(on_wait=[w],
                                                     on_update=[]),
                            bass_priority=inst.bass_priority,
                            bass_scheduled_tick=inst.bass_scheduled_tick,
                            bass_scheduled_proc=inst.bass_scheduled_proc,
                            bass_scheduled_scope=inst.bass_scheduled_scope,
                        )
                        nc.inst_map[ev.name] = ev
                        out.append(ev)
                    si.on_wait = ow[-1:]
                out.append(inst)
            blk.instructions = out
    return n_split


def _get_nc():
    if "nc" not in _NC_CACHE:
        _NC_CACHE["nc"] = _build_bass()
    return _NC_CACHE["nc"]


def _prep_core_inputs(x_core, weight_arrs, n_sc=N_SC, ch_b=CH_B):
    """x_core: [b_core, T, I] fp32 -> the per-core input map."""
    sc_b = 4 * ch_b
    # [sc, ch, b, t, i] -> [sc, t, i, ch*b]
    xr = x_core.reshape(n_sc, 4, ch_b, T_STEPS, I_DIM)
    xf = np.ascontiguousarray(xr.transpose(0, 3, 4, 1, 2)).reshape(
        n_sc, T_STEPS, I_DIM, sc_b)
    xt = np.zeros((n_sc, N_TG, 4, KX, sc_b), _BF16)
    for t in range(T_STEPS):
        tgi, u = divmod(t, 4)
        xt[:, tgi, u, 0:I_DIM, :] = xf[:, t].astype(_BF16)
        xt[:, tgi, u, I_DIM, :] = _BF16(1.0)
    m = {"xt": xt}
    m.update(weight_arrs)
    return m


def _prep_weights(W_ih, W_hh, b_ih, b_hh, W_fc, b_fc):
    W_ih = np.asarray(W_ih, dtype=np.float32)
    W_hh = np.asarray(W_hh, dtype=np.float32)
    b = np.asarray(b_ih, dtype=np.float32) + np.asarray(b_hh, dtype=np.float32)
    W_fc = np.asarray(W_fc, dtype=np.float32)
    b_fc = np.asarray(b_fc, dtype=np.float32)

    wx = np.zeros((4, KX, 128), np.float32)
    wh = np.zeros((4, H_DIM, 128), np.float32)
    wfc = np.zeros((4, H_DIM, H_DIM), np.float32)
    for g in range(4):
        pg = GATE_PERM[g]
        rows = slice(32 * pg, 32 * pg + 32)
        for u in range(4):
            wx[u, 0:I_DIM, 32 * g:32 * g + 32] = W_ih[rows, :].T
            wx[u, I_DIM, 32 * g:32 * g + 32] = b[rows]
            wh[u, :, 32 * g:32 * g + 32] = W_hh[rows, :].T
    for u in range(4):
        wfc[u, :, 0:C_DIM] = W_fc.T
    bfc = np.zeros((128, 1), np.float32)
    for c in range(4):
        bfc[32 * c:32 * c + C_DIM, 0] = b_fc
    return {
        "wx": wx.astype(_BF16),
        "wh": wh.astype(_BF16),
        "wfc": wfc.astype(_BF16),
        "bfc": bfc,
    }


def _run(inputs, trace=False):
    from concourse.bass_utils import run_bass_kernel_spmd

    nc = _get_nc()
    x = np.asarray(inputs["x"], dtype=np.float32)
    w = _prep_weights(inputs["W_ih"], inputs["W_hh"], inputs["b_ih"],
                      inputs["b_hh"], inputs["W_fc"], inputs["b_fc"])
    b_core = B_FULL // N_CORES
    in_maps = [
        _prep_core_inputs(x[i * b_core:(i + 1) * b_core], w)
        for i in range(N_CORES)
    ]
    last_err = None
    for attempt in range(4):
        try:
            res = run_bass_kernel_spmd(
                nc, in_maps, core_ids=list(range(N_CORES)), trace=trace,
            )
            break
        except Exception as e:  # transient device wedges: retry
            last_err = e
            import time as _time
            _time.sleep(3.0)
    else:
        raise last_err
    out = np.concatenate(
        [np.asarray(res.results[i]["out"]) for i in range(N_CORES)], axis=0
    )
    return out, res


def kernel(x, W_ih, W_hh, b_ih, b_hh, W_fc, b_fc):
    out, _ = _run(dict(x=x, W_ih=W_ih, W_hh=W_hh, b_ih=b_ih, b_hh=b_hh,
                       W_fc=W_fc, b_fc=b_fc))
    return out


def _make_jitted(nc, in_maps):
    """Build the shard_map-jitted callable for repeated timed execution
    (mirrors concourse.bass2jax.run_bass_via_pjrt)."""
    import jax
    import numpy as np
    from jax.sharding import Mesh, PartitionSpec
    from jax.experimental.shard_map import shard_map
    import concourse.mybir as mybir
    from concourse import bass2jax
    bass2jax.install_neuronx_cc_hook()
    _bass_exec_p = bass2jax._bass_exec_p

    n_cores = len(in_maps)
    pname = nc.partition_id_tensor.name if nc.partition_id_tensor else None
    in_names, out_names, out_avals, zero_outs = [], [], [], []
    for alloc in nc.m.functions[0].allocations:
        if not isinstance(alloc, mybir.MemoryLocationSet):
            continue
        name = alloc.memorylocations[0].name
        if alloc.kind == "ExternalInput":
            if name != pname:
                in_names.append(name)
        elif alloc.kind == "ExternalOutput":
            out_names.append(name)
            shape = tuple(alloc.tensor_shape)
            dtype = mybir.dt.np(alloc.dtype)
            out_avals.append(jax.core.ShapedArray(shape, dtype))
            zero_outs.append(np.zeros(shape, dtype))
    n_params = len(in_names)
    all_names = in_names + out_names
    if pname is not None:
        all_names = all_names + [pname]

    def _body(*args):
        operands = list(args)
        if pname is not None:
            operands.append(bass2jax.partition_id_tensor())
        outs = _bass_exec_p.bind(
            *operands, out_avals=tuple(out_avals), in_names=tuple(all_names),
            out_names=tuple(out_names), lowering_input_output_aliases=(),
            sim_require_finite=True, sim_require_nnan=True, nc=nc)
        return tuple(outs)

    devices = jax.devices()[:n_cores]
    mesh = Mesh(np.asarray(devices), ("core",))
    sharded = jax.jit(
        shard_map(_body, mesh=mesh,
                  in_specs=(PartitionSpec("core"),) * (n_params + len(out_names)),
                  out_specs=(PartitionSpec("core"),) * len(out_names),
                  check_rep=False),
        keep_unused=True)
    concat_in = [np.concatenate([np.asarray(m[nm]) for m in in_maps], axis=0)
                 for nm in in_names]
    def zeros():
        return [np.zeros((n_cores * z.shape[0], *z.shape[1:]), z.dtype)
                for z in zero_outs]
    return sharded, concat_in, zeros


def _time_kernel(nc, in_maps, iters=30):
    import time
    sharded, concat_in, zeros = _make_jitted(nc, in_maps)
    outs = sharded(*concat_in, *zeros())  # warmup & compile
    for o in outs:
        o.block_until_ready()
    times = []
    for _ in range(iters):
        z = zeros()
        t0 = time.perf_counter()
        outs = sharded(*concat_in, *z)
        for o in outs:
            o.block_until_ready()
        times.append(time.perf_counter() - t0)
    return min(times), sorted(times)[len(times) // 2], outs



# revision 22
# speedup vs baseline: 2.8045x; 2.8045x over previous
"""Trainium2 Bass kernel for nn_AudioRNN (LSTM(13->32, T=25) + FC(32->4), B=65536).

Strategy (pure data parallel over batch, 8 cores x 8192 rows):

  * Host side: x [B,25,13] is cast to bf16 and pre-transposed into the exact
    SBUF layout the TensorEngine needs, with a constant `ones` row appended so
    the LSTM bias rides along in the input-projection matmul.
  * Device side per core: batch is processed as `n_sc` "superchunks" of
    4*ch_b rows, each split into 4 chunks of ch_b rows.  Chunk c lives on
    SBUF/PSUM partition quadrant c (32 partitions = the 32 hidden dims), so
    all per-step tensors (gates, c, h) are lane-aligned for VectorE/ScalarE.
  * Gate pre-activations for one (t, superchunk) live in one PSUM tile
    [128, 4*ch_b]: free-dim bank G holds gate G (order f, i, o, g) so one
    Sigmoid op covers f,i,o and one Tanh covers g.
  * Matmuls use BLOCK-DIAGONAL weights so every matmul writes the full 128
    PSUM partitions (all 4 chunk strips) for one gate: the input projection
    lhsT is [56, 128] with per-chunk blocks [14, 32] (13 input dims + bias
    row), rhs is the pre-transposed x [56, ch_b]; the recurrence lhsT is
    [128, 128] with diagonal blocks W_hh^T [32, 32], rhs is h [128, ch_b].
    8 matmuls of `ch_b` streamed columns per (t, superchunk) -- 4x fewer
    streamed PE columns than a 32x32-PE-tiling formulation.
"""

import numpy as np
import ml_dtypes

I_DIM = 13
H_DIM = 32
C_DIM = 4
T_STEPS = 25
B_FULL = 65536

KX = I_DIM + 1               # 14: 13 input dims + ones row for bias
KP = 4 * KX                  # 56: x partition rows per (t, superchunk)

# free-dim bank order of the gates: f, i, o, g  (sigmoid on banks 0..2, tanh on 3)
# -> PyTorch row-chunk order in W_ih/W_hh is i(0), f(1), g(2), o(3)
GATE_PERM = [1, 0, 3, 2]     # bank G -> pytorch gate chunk index

# production config
N_CORES = 8
CH_B = 512                   # batch rows per chunk (= one PSUM bank of fp32)
N_SC = 4                     # superchunks per core

_BF16 = ml_dtypes.bfloat16

_NC_CACHE = {}


def _build_bass(n_sc=N_SC, ch_b=CH_B, split_waits=True):
    import concourse.bass as bass
    import concourse.mybir as mybir
    from concourse.tile import TileContext
    from concourse.alu_op_type import AluOpType as ALU

    dt = mybir.dt
    AF = mybir.ActivationFunctionType

    sc_b = 4 * ch_b
    b_core = n_sc * sc_b

    nc = bass.Bass("TRN2")

    xt_d = nc.dram_tensor("xt", [n_sc, T_STEPS, KP, ch_b], dt.bfloat16,
                          kind="ExternalInput")
    wx_d = nc.dram_tensor("wx", [KP, 4 * 128], dt.bfloat16, kind="ExternalInput")
    wh_d = nc.dram_tensor("wh", [128, 4 * 128], dt.bfloat16, kind="ExternalInput")
    wfc_d = nc.dram_tensor("wfc", [128, 128], dt.bfloat16, kind="ExternalInput")
    bfc_d = nc.dram_tensor("bfc", [128, 1], dt.float32, kind="ExternalInput")
    # Output stored transposed ([sc, chunk, class, batch]); host transposes
    # back.  The FC weights map chunk c's classes to partitions 4c..4c+4, so
    # one superchunk's whole output is a single contiguous [16, ch_b] DMA.
    out_d = nc.dram_tensor("out", [n_sc, 4 * C_DIM, ch_b], dt.float32,
                           kind="ExternalOutput")

    c_dt = dt.bfloat16  # dtype of the cell state c

    with TileContext(nc) as tc:
        with (
            tc.tile_pool(name="singles", bufs=1) as singles,
            tc.tile_pool(name="xt", bufs=T_STEPS * n_sc) as xt_pool,
            tc.tile_pool(name="sig", bufs=6) as sig_pool,
            tc.tile_pool(name="cell", bufs=8) as cell_pool,
            tc.tile_pool(name="hid", bufs=8) as hid_pool,
            tc.tile_pool(name="tmp", bufs=8) as tmp_pool,
            tc.tile_pool(name="outp", bufs=4) as out_pool,
            tc.tile_pool(name="psum", bufs=2, space="PSUM") as psum_pool,
        ):
            # ---- constants / weights (block-diagonal, one DMA each)
            wx = singles.tile([KP, 4 * 128], dt.bfloat16)
            wh = singles.tile([128, 4 * 128], dt.bfloat16)
            wfc = singles.tile([128, 128], dt.bfloat16)
            bfc = singles.tile([128, 1], dt.float32)
            # Weights go through the gpsimd SWDGE queue so the SP HWDGE queue
            # can start streaming x tiles immediately (SP SEQ pays 565ns per
            # dma_start issue; the first matmuls need wx + xt ASAP).
            nc.gpsimd.dma_start(out=wx, in_=wx_d[:, :])
            nc.gpsimd.dma_start(out=wh, in_=wh_d[:, :])
            nc.gpsimd.dma_start(out=wfc, in_=wfc_d[:, :])
            nc.gpsimd.dma_start(out=bfc, in_=bfc_d[:, :])

            h_prev = [None] * n_sc
            c_prev = [None] * n_sc

            # All n_sc superchunks run as independent interleaved chains
            # rotating through the 2 PSUM gate-tile slots.  With 4 chains,
            # each chain's serial tail (tanh(c) -> h -> recurrence matmul ->
            # sigmoid) has 3 units of slack, so ACT (the bottleneck engine)
            # never waits on it.
            for t in range(T_STEPS):
              for s in range(n_sc):
                xt = xt_pool.tile([KP, ch_b], dt.bfloat16, tag="xt")
                nc.sync.dma_start(out=xt, in_=xt_d[s, t])

                # -- gate pre-activations: one 4-bank PSUM tile
                P = psum_pool.tile([128, 4 * ch_b], dt.float32, tag="gates")
                for g in range(4):
                    nc.tensor.matmul(
                        out=P[:, ch_b * g:ch_b * (g + 1)],
                        lhsT=wx[:, 128 * g:128 * (g + 1)],
                        rhs=xt,
                        start=True,
                        stop=(t == 0),
                        skip_group_check=True,
                    )
                if t > 0:
                    for g in range(4):
                        nc.tensor.matmul(
                            out=P[:, ch_b * g:ch_b * (g + 1)],
                            lhsT=wh[:, 128 * g:128 * (g + 1)],
                            rhs=h_prev[s],
                            start=False,
                            stop=True,
                            skip_group_check=True,
                        )

                # -- activations: ONE tanh op covers all 4 gate banks.
                # Host-side the f,i,o pre-activations are halved, so
                # sigma(x) = (1 + tanh(x/2))/2 = (T+1)/2; the /2 factors are
                # folded into the cell recursion (cell kept DOUBLED: c' = 2c)
                # and into W_hh/W_fc (h' = 2h).
                T4 = sig_pool.tile([128, 4 * ch_b], dt.bfloat16, tag="T4")
                nc.scalar.activation(out=T4, in_=P, func=AF.Tanh)
                Tf = T4[:, 0:ch_b]
                Ti = T4[:, ch_b:2 * ch_b]
                To = T4[:, 2 * ch_b:3 * ch_b]
                Tg = T4[:, 3 * ch_b:4 * ch_b]

                # -- cell update: c' = 2c = (Tf+1)*c'/2 + (Ti+1)*Tg
                Cn = cell_pool.tile([128, ch_b], c_dt, tag="C")
                if t == 0:
                    nc.vector.scalar_tensor_tensor(
                        out=Cn, in0=Ti, scalar=1.0, in1=Tg,
                        op0=ALU.add, op1=ALU.mult)
                else:
                    A = tmp_pool.tile([128, ch_b], c_dt, tag="A")
                    B = tmp_pool.tile([128, ch_b], c_dt, tag="B")
                    nc.vector.scalar_tensor_tensor(
                        out=A, in0=Tf, scalar=1.0, in1=c_prev[s],
                        op0=ALU.add, op1=ALU.mult)
                    nc.vector.scalar_tensor_tensor(
                        out=B, in0=Ti, scalar=1.0, in1=Tg,
                        op0=ALU.add, op1=ALU.mult)
                    nc.vector.scalar_tensor_tensor(
                        out=Cn, in0=A, scalar=0.5, in1=B,
                        op0=ALU.mult, op1=ALU.add)
                # tanh(c) = tanh(0.5 * c') via the activation input scale
                Tc = tmp_pool.tile([128, ch_b], dt.bfloat16, tag="Tc")
                nc.scalar.activation(out=Tc, in_=Cn, func=AF.Tanh, scale=0.5)
                # h' = 2h = (To+1)*tanh(c)   (W_hh, W_fc absorb the 1/2)
                Hn = hid_pool.tile([128, ch_b], dt.bfloat16, tag="H")
                nc.vector.scalar_tensor_tensor(
                    out=Hn, in0=To, scalar=1.0, in1=Tc,
                    op0=ALU.add, op1=ALU.mult)
                c_prev[s] = Cn
                h_prev[s] = Hn

            # -- final FC + bias + store (after the whole t-loop so the FC's
            # PSUM allocations don't break the 2-slot gate-tile rotation at
            # t = T-1, which would serialize the last timestep's units)
            for s in range(n_sc):
                PF = psum_pool.tile([128, ch_b], dt.float32, tag="gates")
                nc.tensor.matmul(
                    out=PF[0:4 * C_DIM, :],
                    lhsT=wfc[:, 0:4 * C_DIM],
                    rhs=h_prev[s],
                    start=True,
                    stop=True,
                    skip_group_check=True,
                )
                Ot = out_pool.tile([4 * C_DIM, ch_b], dt.float32, tag="O")
                nc.vector.tensor_scalar_add(Ot, PF[0:4 * C_DIM, :],
                                            bfc[0:4 * C_DIM, :])
                nc.gpsimd.dma_start(out=out_d[s], in_=Ot)

    if split_waits:
        _split_multi_waits(nc, mybir)
    return nc


def _split_multi_waits(nc, mybir):
    """This walrus build allows only ONE sync-wait command per ISA
    instruction.  Tile sometimes emits 2+ (its wait minimization is not
    transitive across processors).  Hoist all-but-one wait onto standalone
    EventSemaphore instructions injected just before, on the same engine —
    semantically identical (the engine stream blocks at the wait either way).
    """
    n_split = 0
    for fn in nc.m.functions:
        for blk in fn.blocks:
            out = []
            for inst in blk.instructions:
                si = getattr(inst, "sync_info", None)
                ow = list(si.on_wait) if si is not None and si.on_wait else []
                if len(ow) > 1 and inst.opcode == "DMACopy" \
                        and str(inst.engine) in ("EngineType.SP",
                                                 "EngineType.Activation"):
                    raise RuntimeError(
                        f"HWDGE DMA {inst.name} has {len(ow)} waits; "
                        "descriptor waits cannot be split safely")
                if len(ow) > 1:
                    for w in ow[:-1]:
                        n_split += 1
                        ev = mybir.InstEventSemaphore(
                            name=f"splitw-{n_split}-{inst.name}",
                            engine=inst.engine,
                            ins=[],
                            outs=[],
                            sync_info=mybir.SyncInfo(on_wait=[w],
                                                     on_update=[]),
                            bass_priority=inst.bass_priority,
                            bass_scheduled_tick=inst.bass_scheduled_tick,
                            bass_scheduled_proc=inst.bass_scheduled_proc,
                            bass_scheduled_scope=inst.bass_scheduled_scope,
                        )
                        nc.inst_map[ev.name] = ev
                        out.append(ev)
                    si.on_wait = ow[-1:]
                out.append(inst)
            blk.instructions = out
    return n_split


def _get_nc():
    if "nc" not in _NC_CACHE:
        _NC_CACHE["nc"] = _build_bass()
    return _NC_CACHE["nc"]


def _prep_core_inputs(x_core, weight_arrs, n_sc=N_SC, ch_b=CH_B):
    """x_core: [b_core, T, I] fp32 -> the per-core input map."""
    # [sc, ch, b, t, i] -> [sc, t, ch, i, b]
    xr = x_core.reshape(n_sc, 4, ch_b, T_STEPS, I_DIM)
    xf = xr.transpose(0, 3, 1, 4, 2).astype(_BF16)
    xt = np.empty((n_sc, T_STEPS, 4, KX, ch_b), _BF16)
    xt[:, :, :, 0:I_DIM, :] = xf
    xt[:, :, :, I_DIM, :] = _BF16(1.0)
    m = {"xt": np.ascontiguousarray(xt.reshape(n_sc, T_STEPS, KP, ch_b))}
    m.update(weight_arrs)
    return m


def _prep_weights(W_ih, W_hh, b_ih, b_hh, W_fc, b_fc):
    W_ih = np.asarray(W_ih, dtype=np.float32)
    W_hh = np.asarray(W_hh, dtype=np.float32)
    b = np.asarray(b_ih, dtype=np.float32) + np.asarray(b_hh, dtype=np.float32)
    W_fc = np.asarray(W_fc, dtype=np.float32)
    b_fc = np.asarray(b_fc, dtype=np.float32)

    # Block-diagonal lhsT weights: chunk c occupies lhsT rows (K) for its
    # own x/h strip and columns (M) 32c..32c+32 (its PSUM partition strip).
    #
    # Scale folding for the single-tanh gate formulation:
    #  - f,i,o pre-activations are HALVED (sigma(x) = (1+tanh(x/2))/2)
    #  - the recurrence consumes h' = 2h, so W_hh gets another 1/2
    #  - W_fc also consumes h' = 2h -> 1/2
    wx = np.zeros((KP, 4, 128), np.float32)
    wh = np.zeros((128, 4, 128), np.float32)
    wfc = np.zeros((128, 128), np.float32)
    for g in range(4):
        pg = GATE_PERM[g]
        rows = slice(32 * pg, 32 * pg + 32)
        sig_s = 0.5 if g < 3 else 1.0   # banks f,i,o halved; g unscaled
        for c in range(4):
            wx[KX * c:KX * c + I_DIM, g, 32 * c:32 * c + 32] = \
                sig_s * W_ih[rows, :].T
            wx[KX * c + I_DIM, g, 32 * c:32 * c + 32] = sig_s * b[rows]
            wh[32 * c:32 * c + 32, g, 32 * c:32 * c + 32] = \
                (0.5 * sig_s) * W_hh[rows, :].T
    for c in range(4):
        wfc[32 * c:32 * c + H_DIM, C_DIM * c:C_DIM * c + C_DIM] = 0.5 * W_fc.T
    bfc = np.zeros((128, 1), np.float32)
    for c in range(4):
        bfc[C_DIM * c:C_DIM * c + C_DIM, 0] = b_fc
    return {
        "wx": np.ascontiguousarray(wx.reshape(KP, 4 * 128)).astype(_BF16),
        "wh": np.ascontiguousarray(wh.reshape(128, 4 * 128)).astype(_BF16),
        "wfc": wfc.astype(_BF16),
        "bfc": bfc,
    }


def _run(inputs, trace=False):
    from concourse.bass_utils import run_bass_kernel_spmd

    nc = _get_nc()
    x = np.asarray(inputs["x"], dtype=np.float32)
    w = _prep_weights(inputs["W_ih"], inputs["W_hh"], inputs["b_ih"],
                      inputs["b_hh"], inputs["W_fc"], inputs["b_fc"])
    b_core = B_FULL // N_CORES
    in_maps = [
        _prep_core_inputs(x[i * b_core:(i + 1) * b_core], w)
        for i in range(N_CORES)
    ]
    last_err = None
    for attempt in range(4):
        try:
            res = run_bass_kernel_spmd(
                nc, in_maps, core_ids=list(range(N_CORES)), trace=trace,
            )
            break
        except Exception as e:  # transient device wedges: retry
            last_err = e
            import time as _time
            _time.sleep(3.0)
    else:
        raise last_err
    # out per core: [n_sc, 4*C_DIM, ch_b] -> [b_core, C_DIM]
    out = np.concatenate(
        [np.asarray(res.results[i]["out"])
         .reshape(N_SC, 4, C_DIM, CH_B).transpose(0, 1, 3, 2)
         .reshape(-1, C_DIM) for i in range(N_CORES)], axis=0
    )
    return out, res


def kernel(x, W_ih, W_hh, b_ih, b_hh, W_fc, b_fc):
    out, _ = _run(dict(x=x, W_ih=W_ih, W_hh=W_hh, b_ih=b_ih, b_hh=b_hh,
                       W_fc=W_fc, b_fc=b_fc))
    return out
